# revision 33
# baseline (speedup 1.0000x reference)
"""Trainium2 Bass kernel for nn_DecoderBase beam-search decode.

Problem: logits [T=32, B=16, K=10, V=50257] f32 -> beam search with
log_softmax + top-k + regroup per step, then backtrack.

Decomposition
-------------
The per-step top-k over K*V candidates maximizes `score_k - lse_{t,b,k} +
logit`, and the per-beam offset (score - lse) is constant over V. So the
global top-10 per (b) row is contained in the union of per-beam top-10s of
the raw logits, which do NOT depend on the sequential scan state. The
device kernel streams all T*B*K rows once and computes, per row:
  - top candidate values + vocab indices
      * DVE path (2 of 5 row-tiles): per-chunk (16 x ~3142) top-8 via
        Max/MaxIndex
      * GpSimd path (3 of 5 row-tiles): exact top-256 via the Q7 `topk`
        custom op (8 rows per call), balancing DVE (~213us) vs Pool
        (~215us) engine time under the ~365us DMA roofline; the two
        streams are emitted interleaved 4:3 so DMA stays saturated
  - sum(exp(x)) for the log-softmax denominator (ACT Exp + accumulate)
The tiny T-sequential scan (16x10 beams, ~128 candidates per beam per
step) and the backtrack run on host in milliseconds.

Sharding: data-parallel over T across the 8 cores (each row (t,b,k) is
independent in the device phase; T-sharding gives each core one contiguous
DRAM block). The host scan handles all cross-beam coupling.

Sync design: every engine instruction encoding fits exactly ONE sync-wait
(walrus rejects more). The two readers of each streamed chunk are chained
(exp last), making each instruction single-wait by construction; remaining
redundant waits Tile emits (it does not reason transitively) are stripped,
and anything unexpected falls through to Bacc's EventSemaphore splitter.
"""
import os
from contextlib import ExitStack

import numpy as np

import concourse.bacc as bacc
import concourse.mybir as mybir
import concourse.tile as tile
from concourse import library_config
from concourse.bass_utils import run_bass_kernel_spmd
from concourse.tile_rust import add_dep_helper

T, B, K, V = 32, 16, 10, 50257
LENGTH_PENALTY = 0.7
NEG_INF_INIT = -1.0e8

N_CORES = 8
T_PER_CORE = T // N_CORES            # 4
ROWS = T_PER_CORE * B * K            # 640 rows per core
P = 128                              # partitions
CHUNK = 3142                         # 15 full chunks + tail of 3127
NCH = (V + CHUNK - 1) // CHUNK       # 16
NCAND = NCH * 8                      # 128 candidates per row (DVE path)

NTILES = ROWS // P                   # 5
POOL_TILES = 3                       # row-tiles handled by gpsimd topk
DVE_TILES = NTILES - POOL_TILES      # row-tiles handled by DVE max/max_index
DVE_ROWS = DVE_TILES * P             # 384
POOL_ROWS = POOL_TILES * P           # 256
NCALLS = POOL_ROWS // 8              # 32 topk calls (8 rows each)
VPAD = 50304                         # V padded to 16*3144 for topk layout
VSTRIPE = VPAD // 16                 # 3144
TOPK_K = 256
ESTRIDE = 32                         # one 128B granule per exp accumulator


def build_nc(dve_tiles=DVE_TILES, pool_tiles=POOL_TILES, v=V, chunk=CHUNK):
    nch = (v + chunk - 1) // chunk
    dve_rows = dve_tiles * P
    pool_rows = pool_tiles * P
    ncalls = pool_rows // 8
    nc = bacc.Bacc()
    x = nc.declare_dram_parameter(
        "x", [max(dve_rows, 1), v], mybir.dt.float32, isOutput=False)
    if pool_rows:
        xp = nc.declare_dram_parameter(
            "xp", [pool_rows, VPAD], mybir.dt.float32, isOutput=False)
    ovals = nc.declare_dram_parameter(
        "ovals", [max(dve_rows, 1), nch * 8], mybir.dt.float32, isOutput=True)
    oidx = nc.declare_dram_parameter(
        "oidx", [max(dve_rows, 1), nch * 8], mybir.dt.uint32, isOutput=True)
    oes = nc.declare_dram_parameter(
        "oes", [max(dve_rows, 1), nch], mybir.dt.float32, isOutput=True)
    if pool_rows:
        otopk = nc.declare_dram_parameter(
            "otopk", [P, ncalls * 32], mybir.dt.uint32, isOutput=True)
        oesp = nc.declare_dram_parameter(
            "oesp", [P, ncalls], mybir.dt.float32, isOutput=True)

    roles = {}
    with ExitStack() as octx:
        # gpsimd topk needs raw SBTensorHandles (not pool tiles)
        NRING = 4
        tk_in = [octx.enter_context(
            nc.sbuf_tensor(f"tkin{i}", [P, VSTRIPE], mybir.dt.float32))
            for i in range(NRING)] if pool_rows else []
        tk_out = octx.enter_context(nc.sbuf_tensor(
            "tkout", [P, max(ncalls, 1) * 32],
            mybir.dt.uint32)) if pool_rows else None

        with tile.TileContext(nc) as tc, ExitStack() as ctx:
            xpool = ctx.enter_context(tc.tile_pool(name="x", bufs=4))
            spool = ctx.enter_context(tc.tile_pool(name="scr", bufs=2))
            rpool = ctx.enter_context(tc.tile_pool(name="res", bufs=1))
            if pool_rows:
                nc.gpsimd.load_library(library_config.topk)

            # ---------------- DVE path ----------------
            vals8 = rpool.tile(
                [P, max(dve_tiles, 1) * nch * 8], mybir.dt.float32, tag="vals")
            idx8 = rpool.tile(
                [P, max(dve_tiles, 1) * nch * 8], mybir.dt.uint32, tag="idx")
            es = rpool.tile(
                [P, max(dve_tiles, 1) * nch * ESTRIDE], mybir.dt.float32,
                tag="es")
            esp = rpool.tile(
                [P, max(ncalls, 1) * ESTRIDE], mybir.dt.float32, tag="esp")
            # dense staging for the strided exp accumulators (a strided
            # 4B/128B DMA costs ~2.7us in descriptors; a DVE compact is free)
            esd = rpool.tile(
                [P, max(dve_tiles, 1) * nch + max(ncalls, 1)],
                mybir.dt.float32, tag="esd")

            def emit_dve(m, c):
                lo = c * chunk
                L = min(v, lo + chunk) - lo
                o8 = (m * nch + c) * 8
                oe = m * nch + c
                xt = xpool.tile([P, chunk], mybir.dt.float32, tag="xt")
                d = nc.sync.dma_start(
                    xt[:, :L], x[m * P:(m + 1) * P, lo:lo + L])
                mx = nc.vector.max(vals8[:, o8:o8 + 8], xt[:, :L])
                mi = nc.vector.max_index(
                    idx8[:, o8:o8 + 8], vals8[:, o8:o8 + 8], xt[:, :L])
                sc = spool.tile([P, 32], mybir.dt.float32, tag=f"sc_{m}_{c}")
                ex = nc.scalar.activation(
                    sc[:, 0:1].broadcast_to([P, L]), xt[:, :L],
                    mybir.ActivationFunctionType.Exp,
                    accum_out=es[:, oe * ESTRIDE:oe * ESTRIDE + 1])
                # chain only while the slot will be refilled (the chain is
                # what makes the refill DMA single-wait); tail chunks can
                # run exp in parallel with max/max_index
                if (m * nch + c) + 4 < dve_tiles * nch:
                    add_dep_helper(ex.ins, mi.ins, sync=True,
                                   reason="exp is last reader of chunk slot")
                roles[d.ins.name] = "dma_in"
                roles[mx.ins.name] = "max"
                roles[mi.ins.name] = "max_index"
                roles[ex.ins.name] = "exp"
                if c == nch - 1:
                    b8 = m * nch * 8
                    o1 = nc.sync.dma_start(
                        ovals[m * P:(m + 1) * P, :], vals8[:, b8:b8 + nch * 8])
                    o2 = nc.sync.dma_start(
                        oidx[m * P:(m + 1) * P, :], idx8[:, b8:b8 + nch * 8])
                    be = m * nch * ESTRIDE
                    cp = nc.vector.tensor_copy(
                        esd[:, m * nch:(m + 1) * nch],
                        es[:, be:be + nch * ESTRIDE:ESTRIDE])
                    o3 = nc.sync.dma_start(
                        oes[m * P:(m + 1) * P, :], esd[:, m * nch:(m + 1) * nch])
                    roles[o1.ins.name] = roles[o2.ins.name] = "dma_out_dve"
                    roles[cp.ins.name] = "escopy"
                    roles[o3.ins.name] = "dma_out_dve"

            def emit_pool(call):
                r0 = call * 8  # row offset within xp
                buf = tk_in[call % NRING]
                # one DMA: 8 padded rows -> [128, 3144], partition 16t+j
                # holds xp[r0+t, j*3144:(j+1)*3144]
                src = xp[r0:r0 + 8, :].rearrange("r (p c) -> (r p) c", p=16)
                d = nc.sync.dma_start(buf[:], src)
                tk = nc.gpsimd.topk(
                    tk_out[:, call * 32:(call + 1) * 32], buf[:],
                    tokens=8, vocab_size=VPAD, k=TOPK_K)
                sc = spool.tile([P, 32], mybir.dt.float32, tag=f"scp_{call}")
                ex = nc.scalar.activation(
                    sc[:, 0:1].broadcast_to([P, VSTRIPE]), buf[:],
                    mybir.ActivationFunctionType.Exp,
                    accum_out=esp[:, call * ESTRIDE:call * ESTRIDE + 1])
                if call + NRING < ncalls:
                    add_dep_helper(ex.ins, tk.ins, sync=True,
                                   reason="exp is last reader of topk slot")
                roles[d.ins.name] = "dma_in"
                roles[tk.ins.name] = "topk"
                roles[ex.ins.name] = "exp_pool"
                half = ncalls // 2
                if call + 1 in (half, ncalls):
                    fl = 0 if call + 1 == half else half
                    o4 = nc.sync.dma_start(
                        otopk[:, fl * 32:(call + 1) * 32],
                        tk_out[:, fl * 32:(call + 1) * 32])
                    roles[o4.ins.name] = "dma_out_pool"
                if call == ncalls - 1:
                    cp = nc.vector.tensor_copy(
                        esd[:, dve_tiles * nch:dve_tiles * nch + ncalls],
                        esp[:, :ncalls * ESTRIDE:ESTRIDE])
                    o5 = nc.sync.dma_start(
                        oesp[:, :],
                        esd[:, dve_tiles * nch:dve_tiles * nch + ncalls])
                    roles[cp.ins.name] = "escopy"
                    roles[o5.ins.name] = "dma_out_dve"

            # interleave the two paths so DMA pressure and both compute
            # engines stay evenly fed (64 dve chunks : 48 pool calls = 4:3)
            dve_items = [(m, c) for m in range(dve_tiles) for c in range(nch)]
            pool_items = list(range(ncalls))
            di = pi = 0
            while di < len(dve_items) or pi < len(pool_items):
                for _ in range(4):
                    if di < len(dve_items):
                        emit_dve(*dve_items[di])
                        di += 1
                for _ in range(3):
                    if pi < len(pool_items):
                        emit_pool(pool_items[pi])
                        pi += 1

        # ---- single-wait legalization (see module docstring) ----
        _PREF = {"dma_in": ("Activation", "DMAHW", "DMASW", "DVE", "Pool"),
                 "max": ("DMAHW", "DMASW"),
                 "max_index": ("DVE",),
                 "topk": ("DMAHW", "DMASW"),
                 "exp": ("DVE",),
                 "exp_pool": ("Pool",),
                 "escopy": ("Activation",),
                 "dma_out_dve": ("DVE",),
                 "dma_out_pool": ("Pool",),
                 "dma_out_act": ("Activation",)}
        for blk in nc.m.functions[0].blocks:
            for inst in blk.instructions:
                si = inst.sync_info
                role = roles.get(inst.name)
                if si is None or role is None:
                    continue
                waits = list(si.on_wait)
                if len(waits) > 1:
                    for p in _PREF[role]:
                        kept = [w for w in waits if w.ant_name.startswith(p)]
                        if kept:
                            break
                    if len(kept) == 1:
                        inst.sync_info = mybir.SyncInfo(
                            on_wait=kept, on_update=list(si.on_update))
        nc.compile()
    return nc


_NC_CACHE = {}


def _get_nc():
    if "nc" not in _NC_CACHE:
        _NC_CACHE["nc"] = build_nc()
    return _NC_CACHE["nc"]


def _device_phase(logits):
    """Full logits [T,B,K,V] -> vals [T*B*K, NC] f32,
    gidx [T*B*K, NC] int64 (global vocab ids, NC=128, sorted desc),
    lse [T*B*K] f32."""
    nc = _get_nc()
    flat = logits.reshape(T * B * K, V)
    in_maps = []
    for c in range(N_CORES):
        block = flat[c * ROWS:(c + 1) * ROWS]
        xm = np.ascontiguousarray(block[:DVE_ROWS])
        xpm = np.full((POOL_ROWS, VPAD), -1.0e30, np.float32)
        xpm[:, :V] = block[DVE_ROWS:]
        in_maps.append({"x": xm, "xp": xpm})
    res = run_bass_kernel_spmd(nc, in_maps, list(range(N_CORES)))
    _NC_CACHE["last_results"] = res

    NC = NCAND
    nrows = T * B * K
    vals = np.empty((nrows, NC), np.float32)
    gidx = np.empty((nrows, NC), np.int64)
    esum = np.empty(nrows, np.float64)
    base = (np.arange(NCH, dtype=np.int64) * CHUNK)[None, :, None]
    for c in range(N_CORES):
        r = res.results[c]
        lo = c * ROWS
        # DVE rows
        vals[lo:lo + DVE_ROWS] = r["ovals"]
        lidx = r["oidx"].reshape(DVE_ROWS, NCH, 8).astype(np.int64) + base
        gidx[lo:lo + DVE_ROWS] = lidx.reshape(DVE_ROWS, NCH * 8)
        esum[lo:lo + DVE_ROWS] = r["oes"].astype(np.float64).sum(-1)
        # topk rows: otopk [128, ncalls*32]; token t of call -> partitions
        # 16t..16t+16, cols [call*32, call*32+16) vals / +16..32 idx,
        # globally ascending in row-major flatten order
        tk = r["otopk"].reshape(8, 16, NCALLS, 32).transpose(2, 0, 1, 3)
        # -> [call, tok, 16, 32]
        tvals = tk[:, :, :, :16].reshape(NCALLS, 8, 256).view(np.float32)
        tidx = tk[:, :, :, 16:].reshape(NCALLS, 8, 256).astype(np.int64)
        # top-NC, descending
        tvals = tvals[:, :, -NC:][:, :, ::-1].reshape(POOL_ROWS, NC)
        tidx = tidx[:, :, -NC:][:, :, ::-1].reshape(POOL_ROWS, NC)
        vals[lo + DVE_ROWS:lo + ROWS] = tvals
        gidx[lo + DVE_ROWS:lo + ROWS] = tidx
        ep = r["oesp"].reshape(8, 16, NCALLS).transpose(2, 0, 1)  # call,tok,16
        esum[lo + DVE_ROWS:lo + ROWS] = (
            ep.astype(np.float64).sum(-1).reshape(POOL_ROWS))
    lse = np.log(esum).astype(np.float32)
    return (vals.reshape(T, B, K, NC), gidx.reshape(T, B, K, NC),
            lse.reshape(T, B, K))


def _host_scan(vals, gidx, lse, eos_id):
    """Beam-search scan + backtrack, replicating the reference semantics."""
    flag = np.zeros((B, K), bool)
    score = np.full((B, K), NEG_INF_INIT, np.float32)
    score[:, 0] = 0.0
    now_length = np.zeros((B, K), np.float32)

    w_ids = np.empty((T, B, K), np.int64)
    regroups = np.empty((T, B, K), np.int64)
    eosmets = np.empty((T, B, K), bool)

    karange = np.arange(K)
    for t in range(T):
        valid = (~flag).astype(np.float32)                    # [B,K]
        w = vals[t] - lse[t][:, :, None]                      # [B,K,NC]
        cand = score[:, :, None] + w * valid[:, :, None]
        denom = ((now_length + valid + 1e-9) ** LENGTH_PENALTY
                 ).astype(np.float32)
        new_score = cand / denom[:, :, None]
        flat = karange[None, :, None] * V + gidx[t]           # [B,K,NC]

        # finished beams: only their v=0 candidate can be selected (the
        # min*10 penalty pushes v>=1 below every unpenalized candidate);
        # new_score = score/denom, cand = score there.
        ns = np.where(valid[:, :, None] > 0, new_score, -np.inf)
        cd = np.where(valid[:, :, None] > 0, cand, 0.0)
        synth_ns = np.where(valid > 0, -np.inf, score / denom)
        synth_cd = score
        synth_flat = np.broadcast_to(karange[None, :] * V, (B, K))

        ns_all = np.concatenate([ns.reshape(B, -1), synth_ns], axis=1)
        cd_all = np.concatenate([cd.reshape(B, -1), synth_cd], axis=1)
        fl_all = np.concatenate([flat.reshape(B, -1), synth_flat], axis=1)

        sel = np.empty((B, K), np.int64)
        for b in range(B):
            order = np.lexsort((fl_all[b], -ns_all[b]))  # value desc, idx asc
            sel[b] = order[:K]
        score = np.take_along_axis(cd_all, sel, axis=1).astype(np.float32)
        flat_sel = np.take_along_axis(fl_all, sel, axis=1)

        regroup = flat_sel // V
        w_id = flat_sel % V
        flag_g = np.take_along_axis(flag, regroup, axis=1)
        now_length = np.take_along_axis(now_length, regroup, axis=1) + valid
        eosmets[t] = flag_g
        flag = flag_g | ((w_id == eos_id) & (valid > 0.0))
        w_ids[t] = w_id
        regroups[t] = regroup

    # backtrack (reverse gather chain)
    now_index = np.tile(np.arange(K)[None, :], (B, 1))
    wo = np.empty((T, B, K), np.int64)
    em = np.empty((T, B, K), bool)
    for t in range(T - 1, -1, -1):
        wo[t] = np.take_along_axis(w_ids[t], now_index, axis=1)
        em[t] = np.take_along_axis(eosmets[t], now_index, axis=1)
        now_index = np.take_along_axis(regroups[t], now_index, axis=1)
    back_eosmet = 1 - em.astype(np.int32)
    wo = (wo * back_eosmet).astype(np.int32)
    length = back_eosmet.sum(axis=0).astype(np.int32)
    return wo, length, score


def kernel(logits, eos_id):
    logits = np.ascontiguousarray(np.asarray(logits, dtype=np.float32))
    assert logits.shape == (T, B, K, V), logits.shape
    eos = int(np.asarray(eos_id))
    vals, gidx, lse = _device_phase(logits)
    return _host_scan(vals, gidx, lse, eos)


# revision 40
# speedup vs baseline: 1.0196x; 1.0196x over previous
"""Trainium2 Bass kernel for nn_DecoderBase beam-search decode.

Problem: logits [T=32, B=16, K=10, V=50257] f32 -> beam search with
log_softmax + top-k + regroup per step, then backtrack.

Decomposition
-------------
The per-step top-k over K*V candidates maximizes `score_k - lse_{t,b,k} +
logit`, and the per-beam offset (score - lse) is constant over V. So the
global top-10 per (b) row is contained in the union of per-beam top-10s of
the raw logits, which do NOT depend on the sequential scan state. The
device kernel streams all T*B*K rows once and computes, per row:
  - top candidate values + vocab indices
      * DVE path (2 of 5 row-tiles): per-chunk (16 x ~3142) top-8 via
        Max/MaxIndex
      * GpSimd path (3 of 5 row-tiles): exact top-256 via the Q7 `topk`
        custom op (8 rows per call), balancing DVE (~213us) vs Pool
        (~215us) engine time under the ~365us DMA roofline; the two
        streams are emitted interleaved 4:3 so DMA stays saturated
  - sum(exp(x)) for the log-softmax denominator (ACT Exp + accumulate)
The tiny T-sequential scan (16x10 beams, ~128 candidates per beam per
step) and the backtrack run on host in milliseconds.

Sharding: data-parallel over T across the 8 cores (each row (t,b,k) is
independent in the device phase; T-sharding gives each core one contiguous
DRAM block). The host scan handles all cross-beam coupling.

Sync design: every engine instruction encoding fits exactly ONE sync-wait
(walrus rejects more). The two readers of each streamed chunk are chained
(exp last), making each instruction single-wait by construction; remaining
redundant waits Tile emits (it does not reason transitively) are stripped,
and anything unexpected falls through to Bacc's EventSemaphore splitter.
"""
import os
from contextlib import ExitStack

import numpy as np

import concourse.bacc as bacc
import concourse.mybir as mybir
import concourse.tile as tile
from concourse import library_config
from concourse.bass_utils import run_bass_kernel_spmd
from concourse.tile_rust import add_dep_helper

T, B, K, V = 32, 16, 10, 50257
LENGTH_PENALTY = 0.7
NEG_INF_INIT = -1.0e8

N_CORES = 8
T_PER_CORE = T // N_CORES            # 4
ROWS = T_PER_CORE * B * K            # 640 rows per core
P = 128                              # partitions
CHUNK = 3142                         # 15 full chunks + tail of 3127
NCH = (V + CHUNK - 1) // CHUNK       # 16
NCAND = NCH * 8                      # 128 candidates per row (DVE path)

NTILES = ROWS // P                   # 5
POOL_TILES = 3                       # row-tiles handled by gpsimd topk
DVE_TILES = NTILES - POOL_TILES      # row-tiles handled by DVE max/max_index
DVE_ROWS = DVE_TILES * P             # 384
POOL_ROWS = POOL_TILES * P           # 256
NCALLS = POOL_ROWS // 8              # 32 topk calls (8 rows each)
VPAD = 50304                         # V padded to 16*3144 for topk layout
VSTRIPE = VPAD // 16                 # 3144
TOPK_K = 256
ESTRIDE = 32                         # one 128B granule per exp accumulator


def build_nc(dve_tiles=DVE_TILES, pool_tiles=POOL_TILES, v=V, chunk=CHUNK):
    nch = (v + chunk - 1) // chunk
    dve_rows = dve_tiles * P
    pool_rows = pool_tiles * P
    ncalls = pool_rows // 8
    nc = bacc.Bacc()
    x = nc.declare_dram_parameter(
        "x", [max(dve_rows, 1), v], mybir.dt.float32, isOutput=False)
    if pool_rows:
        xp = nc.declare_dram_parameter(
            "xp", [pool_rows, VPAD], mybir.dt.float32, isOutput=False)
    ovals = nc.declare_dram_parameter(
        "ovals", [max(dve_rows, 1), nch * 8], mybir.dt.float32, isOutput=True)
    oidx = nc.declare_dram_parameter(
        "oidx", [max(dve_rows, 1), nch * 8], mybir.dt.uint32, isOutput=True)
    oes = nc.declare_dram_parameter(
        "oes", [max(dve_rows, 1), nch], mybir.dt.float32, isOutput=True)
    if pool_rows:
        otopk = nc.declare_dram_parameter(
            "otopk", [P, ncalls * 32], mybir.dt.uint32, isOutput=True)
        oesp = nc.declare_dram_parameter(
            "oesp", [P, ncalls], mybir.dt.float32, isOutput=True)

    roles = {}
    dma_exp = {}   # input dma -> its exp instruction
    slot_prev = {}  # input dma -> dma whose slot it reuses
    dve_hist, pool_hist = [], []
    with ExitStack() as octx:
        # gpsimd topk needs raw SBTensorHandles (not pool tiles)
        NRING = 4
        tk_in = [octx.enter_context(
            nc.sbuf_tensor(f"tkin{i}", [P, VSTRIPE], mybir.dt.float32))
            for i in range(NRING)] if pool_rows else []
        tk_out = octx.enter_context(nc.sbuf_tensor(
            "tkout", [P, max(ncalls, 1) * 32],
            mybir.dt.uint32)) if pool_rows else None

        with tile.TileContext(nc) as tc, ExitStack() as ctx:
            xpool = ctx.enter_context(tc.tile_pool(name="x", bufs=4))
            spool = ctx.enter_context(tc.tile_pool(name="scr", bufs=2))
            rpool = ctx.enter_context(tc.tile_pool(name="res", bufs=1))
            if pool_rows:
                nc.gpsimd.load_library(library_config.topk)

            # ---------------- DVE path ----------------
            vals8 = rpool.tile(
                [P, max(dve_tiles, 1) * nch * 8], mybir.dt.float32, tag="vals")
            idx8 = rpool.tile(
                [P, max(dve_tiles, 1) * nch * 8], mybir.dt.uint32, tag="idx")
            es = rpool.tile(
                [P, max(dve_tiles, 1) * nch * ESTRIDE], mybir.dt.float32,
                tag="es")
            esp = rpool.tile(
                [P, max(ncalls, 1) * ESTRIDE], mybir.dt.float32, tag="esp")
            # dense staging for the strided exp accumulators (a strided
            # 4B/128B DMA costs ~2.7us in descriptors; a DVE compact is free)
            esd = rpool.tile(
                [P, max(dve_tiles, 1) * nch + max(ncalls, 1)],
                mybir.dt.float32, tag="esd")

            def emit_dve(m, c):
                lo = c * chunk
                L = min(v, lo + chunk) - lo
                o8 = (m * nch + c) * 8
                oe = m * nch + c
                xt = xpool.tile([P, chunk], mybir.dt.float32, tag="xt")
                d = nc.sync.dma_start(
                    xt[:, :L], x[m * P:(m + 1) * P, lo:lo + L])
                mx = nc.vector.max(vals8[:, o8:o8 + 8], xt[:, :L])
                mi = nc.vector.max_index(
                    idx8[:, o8:o8 + 8], vals8[:, o8:o8 + 8], xt[:, :L])
                sc = spool.tile([P, 32], mybir.dt.float32, tag=f"sc_{m}_{c}")
                ex = nc.scalar.activation(
                    sc[:, 0:1].broadcast_to([P, L]), xt[:, :L],
                    mybir.ActivationFunctionType.Exp,
                    accum_out=es[:, oe * ESTRIDE:oe * ESTRIDE + 1])
                # chain only while the slot will be refilled (the chain is
                # what makes the refill DMA single-wait); tail chunks can
                # run exp in parallel with max/max_index
                if (m * nch + c) + 4 < dve_tiles * nch:
                    add_dep_helper(ex.ins, mi.ins, sync=True,
                                   reason="exp is last reader of chunk slot")
                roles[d.ins.name] = "dma_in"
                roles[mx.ins.name] = "max"
                roles[mi.ins.name] = "max_index"
                roles[ex.ins.name] = "exp"
                dma_exp[d.ins.name] = ex.ins.name
                dve_hist.append(d.ins.name)
                if len(dve_hist) > 4:
                    slot_prev[d.ins.name] = dve_hist[-5]
                if c == nch - 1:
                    b8 = m * nch * 8
                    o1 = nc.scalar.dma_start(
                        ovals[m * P:(m + 1) * P, :], vals8[:, b8:b8 + nch * 8])
                    o2 = nc.scalar.dma_start(
                        oidx[m * P:(m + 1) * P, :], idx8[:, b8:b8 + nch * 8])
                    be = m * nch * ESTRIDE
                    cp = nc.vector.tensor_copy(
                        esd[:, m * nch:(m + 1) * nch],
                        es[:, be:be + nch * ESTRIDE:ESTRIDE])
                    roles[o1.ins.name] = roles[o2.ins.name] = "dma_out_dve"
                    roles[cp.ins.name] = "escopy"
                    if m == dve_tiles - 1:
                        # single merged oes out for all dve tiles
                        dst = oes[:, :].rearrange("(m p) c -> p m c", p=P)
                        srcv = esd[:, :dve_tiles * nch].rearrange(
                            "p (m c) -> p m c", c=nch)
                        o3 = nc.scalar.dma_start(dst, srcv)
                        roles[o3.ins.name] = "dma_out_dve"

            def emit_pool(call):
                r0 = call * 8  # row offset within xp
                buf = tk_in[call % NRING]
                # one DMA: 8 padded rows -> [128, 3144], partition 16t+j
                # holds xp[r0+t, j*3144:(j+1)*3144]
                src = xp[r0:r0 + 8, :].rearrange("r (p c) -> (r p) c", p=16)
                d = nc.sync.dma_start(buf[:], src)
                tk = nc.gpsimd.topk(
                    tk_out[:, call * 32:(call + 1) * 32], buf[:],
                    tokens=8, vocab_size=VPAD, k=TOPK_K)
                sc = spool.tile([P, 32], mybir.dt.float32, tag=f"scp_{call}")
                ex = nc.scalar.activation(
                    sc[:, 0:1].broadcast_to([P, VSTRIPE]), buf[:],
                    mybir.ActivationFunctionType.Exp,
                    accum_out=esp[:, call * ESTRIDE:call * ESTRIDE + 1])
                if call + NRING < ncalls:
                    add_dep_helper(ex.ins, tk.ins, sync=True,
                                   reason="exp is last reader of topk slot")
                roles[d.ins.name] = "dma_in"
                roles[tk.ins.name] = "topk"
                roles[ex.ins.name] = "exp_pool"
                dma_exp[d.ins.name] = ex.ins.name
                pool_hist.append(d.ins.name)
                if len(pool_hist) > NRING:
                    slot_prev[d.ins.name] = pool_hist[-NRING - 1]
                half = ncalls // 2
                if call + 1 in (half, ncalls):
                    fl = 0 if call + 1 == half else half
                    o4 = nc.scalar.dma_start(
                        otopk[:, fl * 32:(call + 1) * 32],
                        tk_out[:, fl * 32:(call + 1) * 32])
                    roles[o4.ins.name] = "dma_out_pool"
                if call == ncalls - 1:
                    cp = nc.vector.tensor_copy(
                        esd[:, dve_tiles * nch:dve_tiles * nch + ncalls],
                        esp[:, :ncalls * ESTRIDE:ESTRIDE])
                    o5 = nc.scalar.dma_start(
                        oesp[:, :],
                        esd[:, dve_tiles * nch:dve_tiles * nch + ncalls])
                    roles[cp.ins.name] = "escopy"
                    roles[o5.ins.name] = "dma_out_dve"

            # interleave the two paths so DMA pressure and both compute
            # engines stay evenly fed (64 dve chunks : 48 pool calls = 4:3)
            dve_items = [(m, c) for m in range(dve_tiles) for c in range(nch)]
            pool_items = list(range(ncalls))
            di = pi = 0
            while di < len(dve_items) or pi < len(pool_items):
                for _ in range(4):
                    if di < len(dve_items):
                        emit_dve(*dve_items[di])
                        di += 1
                for _ in range(3):
                    if pi < len(pool_items):
                        emit_pool(pool_items[pi])
                        pi += 1

        # ---- single-wait legalization (see module docstring) ----
        _PREF = {"dma_in": ("Activation", "DMAHW", "DMASW", "DVE", "Pool"),
                 "max": ("DMAHW", "DMASW"),
                 "max_index": ("DVE",),
                 "topk": ("DMAHW", "DMASW"),
                 "exp": ("DVE",),
                 "exp_pool": ("Pool",),
                 "escopy": ("Activation",),
                 "dma_out_dve": ("DVE",),
                 "dma_out_pool": ("Pool",),
                 "dma_out_act": ("Activation",)}
        for blk in nc.m.functions[0].blocks:
            for inst in blk.instructions:
                si = inst.sync_info
                role = roles.get(inst.name)
                if si is None or role is None:
                    continue
                waits = list(si.on_wait)
                if len(waits) > 1:
                    for p in _PREF[role]:
                        kept = [w for w in waits if w.ant_name.startswith(p)]
                        if kept:
                            break
                    if len(kept) == 1:
                        inst.sync_info = mybir.SyncInfo(
                            on_wait=kept, on_update=list(si.on_update))

        # ---- queue-WAW coverage fixup ----
        # Tile assigns HWDGE queues in SCHEDULED order, so an input DMA's
        # same-queue predecessor is not statically known. Each input keeps
        # a single ACT wait; exp retirement at tick t implies every input
        # whose exp tick <= t has completed (exp is, or follows, the last
        # reader of its chunk). Raise each input's ACT wait to also cover
        # its queue predecessor's exp tick.
        act_tick = {}
        ticks = 0
        for blk in nc.m.functions[0].blocks:
            for inst in blk.instructions:
                if inst.engine == mybir.EngineType.Activation and \
                        inst.__class__.__name__ == "InstActivation":
                    ticks += 1
                    act_tick[inst.name] = ticks
        act_sem = None
        last_on_queue = {}
        for blk in nc.m.functions[0].blocks:
            for inst in blk.instructions:
                si = inst.sync_info
                if roles.get(inst.name) != "dma_in" or si is None:
                    continue
                q = si.on_update[0].ant_name
                prev = last_on_queue.get(q)
                last_on_queue[q] = inst.name
                aw = [w for w in si.on_wait
                      if w.ant_name.startswith("Activation")]
                if aw and act_sem is None:
                    act_sem = (aw[0].id, aw[0].ant_name)
                if prev is None:
                    continue
                need = act_tick[dma_exp[prev]]
                sp = slot_prev.get(inst.name)
                if sp is not None:
                    need = max(need, act_tick[dma_exp[sp]])
                have = aw[0].wait_value if aw else 0
                if need > have:
                    assert act_sem is not None
                    w = mybir.SyncWait(
                        sync_type="semaphore", id=act_sem[0],
                        ant_name=act_sem[1], wait_mode="sem-ge-imm",
                        wait_value=need, wait_reg=None)
                    inst.sync_info = mybir.SyncInfo(
                        on_wait=[w], on_update=list(si.on_update))
        nc.compile()
    return nc


_NC_CACHE = {}


def _get_nc():
    if "nc" not in _NC_CACHE:
        _NC_CACHE["nc"] = build_nc()
    return _NC_CACHE["nc"]


def _device_phase(logits):
    """Full logits [T,B,K,V] -> vals [T*B*K, NC] f32,
    gidx [T*B*K, NC] int64 (global vocab ids, NC=128, sorted desc),
    lse [T*B*K] f32."""
    nc = _get_nc()
    flat = logits.reshape(T * B * K, V)
    in_maps = []
    for c in range(N_CORES):
        block = flat[c * ROWS:(c + 1) * ROWS]
        xm = np.ascontiguousarray(block[:DVE_ROWS])
        xpm = np.full((POOL_ROWS, VPAD), -1.0e30, np.float32)
        xpm[:, :V] = block[DVE_ROWS:]
        in_maps.append({"x": xm, "xp": xpm})
    res = run_bass_kernel_spmd(nc, in_maps, list(range(N_CORES)))
    _NC_CACHE["last_results"] = res

    NC = NCAND
    nrows = T * B * K
    vals = np.empty((nrows, NC), np.float32)
    gidx = np.empty((nrows, NC), np.int64)
    esum = np.empty(nrows, np.float64)
    base = (np.arange(NCH, dtype=np.int64) * CHUNK)[None, :, None]
    for c in range(N_CORES):
        r = res.results[c]
        lo = c * ROWS
        # DVE rows
        vals[lo:lo + DVE_ROWS] = r["ovals"]
        lidx = r["oidx"].reshape(DVE_ROWS, NCH, 8).astype(np.int64) + base
        gidx[lo:lo + DVE_ROWS] = lidx.reshape(DVE_ROWS, NCH * 8)
        esum[lo:lo + DVE_ROWS] = r["oes"].astype(np.float64).sum(-1)
        # topk rows: otopk [128, ncalls*32]; token t of call -> partitions
        # 16t..16t+16, cols [call*32, call*32+16) vals / +16..32 idx,
        # globally ascending in row-major flatten order
        tk = r["otopk"].reshape(8, 16, NCALLS, 32).transpose(2, 0, 1, 3)
        # -> [call, tok, 16, 32]
        tvals = tk[:, :, :, :16].reshape(NCALLS, 8, 256).view(np.float32)
        tidx = tk[:, :, :, 16:].reshape(NCALLS, 8, 256).astype(np.int64)
        # top-NC, descending
        tvals = tvals[:, :, -NC:][:, :, ::-1].reshape(POOL_ROWS, NC)
        tidx = tidx[:, :, -NC:][:, :, ::-1].reshape(POOL_ROWS, NC)
        vals[lo + DVE_ROWS:lo + ROWS] = tvals
        gidx[lo + DVE_ROWS:lo + ROWS] = tidx
        ep = r["oesp"].reshape(8, 16, NCALLS).transpose(2, 0, 1)  # call,tok,16
        esum[lo + DVE_ROWS:lo + ROWS] = (
            ep.astype(np.float64).sum(-1).reshape(POOL_ROWS))
    lse = np.log(esum).astype(np.float32)
    return (vals.reshape(T, B, K, NC), gidx.reshape(T, B, K, NC),
            lse.reshape(T, B, K))


def _host_scan(vals, gidx, lse, eos_id):
    """Beam-search scan + backtrack, replicating the reference semantics."""
    flag = np.zeros((B, K), bool)
    score = np.full((B, K), NEG_INF_INIT, np.float32)
    score[:, 0] = 0.0
    now_length = np.zeros((B, K), np.float32)

    w_ids = np.empty((T, B, K), np.int64)
    regroups = np.empty((T, B, K), np.int64)
    eosmets = np.empty((T, B, K), bool)

    karange = np.arange(K)
    for t in range(T):
        valid = (~flag).astype(np.float32)                    # [B,K]
        w = vals[t] - lse[t][:, :, None]                      # [B,K,NC]
        cand = score[:, :, None] + w * valid[:, :, None]
        denom = ((now_length + valid + 1e-9) ** LENGTH_PENALTY
                 ).astype(np.float32)
        new_score = cand / denom[:, :, None]
        flat = karange[None, :, None] * V + gidx[t]           # [B,K,NC]

        # finished beams: only their v=0 candidate can be selected (the
        # min*10 penalty pushes v>=1 below every unpenalized candidate);
        # new_score = score/denom, cand = score there.
        ns = np.where(valid[:, :, None] > 0, new_score, -np.inf)
        cd = np.where(valid[:, :, None] > 0, cand, 0.0)
        synth_ns = np.where(valid > 0, -np.inf, score / denom)
        synth_cd = score
        synth_flat = np.broadcast_to(karange[None, :] * V, (B, K))

        ns_all = np.concatenate([ns.reshape(B, -1), synth_ns], axis=1)
        cd_all = np.concatenate([cd.reshape(B, -1), synth_cd], axis=1)
        fl_all = np.concatenate([flat.reshape(B, -1), synth_flat], axis=1)

        sel = np.empty((B, K), np.int64)
        for b in range(B):
            order = np.lexsort((fl_all[b], -ns_all[b]))  # value desc, idx asc
            sel[b] = order[:K]
        score = np.take_along_axis(cd_all, sel, axis=1).astype(np.float32)
        flat_sel = np.take_along_axis(fl_all, sel, axis=1)

        regroup = flat_sel // V
        w_id = flat_sel % V
        flag_g = np.take_along_axis(flag, regroup, axis=1)
        now_length = np.take_along_axis(now_length, regroup, axis=1) + valid
        eosmets[t] = flag_g
        flag = flag_g | ((w_id == eos_id) & (valid > 0.0))
        w_ids[t] = w_id
        regroups[t] = regroup

    # backtrack (reverse gather chain)
    now_index = np.tile(np.arange(K)[None, :], (B, 1))
    wo = np.empty((T, B, K), np.int64)
    em = np.empty((T, B, K), bool)
    for t in range(T - 1, -1, -1):
        wo[t] = np.take_along_axis(w_ids[t], now_index, axis=1)
        em[t] = np.take_along_axis(eosmets[t], now_index, axis=1)
        now_index = np.take_along_axis(regroups[t], now_index, axis=1)
    back_eosmet = 1 - em.astype(np.int32)
    wo = (wo * back_eosmet).astype(np.int32)
    length = back_eosmet.sum(axis=0).astype(np.int32)
    return wo, length, score


def kernel(logits, eos_id):
    logits = np.ascontiguousarray(np.asarray(logits, dtype=np.float32))
    assert logits.shape == (T, B, K, V), logits.shape
    eos = int(np.asarray(eos_id))
    vals, gidx, lse = _device_phase(logits)
    return _host_scan(vals, gidx, lse, eos)


# revision 41
# speedup vs baseline: 1.0217x; 1.0020x over previous
"""Trainium2 Bass kernel for nn_DecoderBase beam-search decode.

Problem: logits [T=32, B=16, K=10, V=50257] f32 -> beam search with
log_softmax + top-k + regroup per step, then backtrack.

Decomposition
-------------
The per-step top-k over K*V candidates maximizes `score_k - lse_{t,b,k} +
logit`, and the per-beam offset (score - lse) is constant over V. So the
global top-10 per (b) row is contained in the union of per-beam top-10s of
the raw logits, which do NOT depend on the sequential scan state. The
device kernel streams all T*B*K rows once and computes, per row:
  - top candidate values + vocab indices
      * DVE path (2 of 5 row-tiles): per-chunk (16 x ~3142) top-8 via
        Max/MaxIndex
      * GpSimd path (3 of 5 row-tiles): exact top-256 via the Q7 `topk`
        custom op (8 rows per call), balancing DVE (~213us) vs Pool
        (~215us) engine time under the ~365us DMA roofline; the two
        streams are emitted interleaved 4:3 so DMA stays saturated
  - sum(exp(x)) for the log-softmax denominator (ACT Exp + accumulate)
The tiny T-sequential scan (16x10 beams, ~128 candidates per beam per
step) and the backtrack run on host in milliseconds.

Sharding: data-parallel over T across the 8 cores (each row (t,b,k) is
independent in the device phase; T-sharding gives each core one contiguous
DRAM block). The host scan handles all cross-beam coupling.

Sync design: every engine instruction encoding fits exactly ONE sync-wait
(walrus rejects more). The two readers of each streamed chunk are chained
(exp last), making each instruction single-wait by construction; remaining
redundant waits Tile emits (it does not reason transitively) are stripped,
and anything unexpected falls through to Bacc's EventSemaphore splitter.
"""
import os
from contextlib import ExitStack

import numpy as np

import concourse.bacc as bacc
import concourse.mybir as mybir
import concourse.tile as tile
from concourse import library_config
from concourse.bass_utils import run_bass_kernel_spmd
from concourse.tile_rust import add_dep_helper

T, B, K, V = 32, 16, 10, 50257
LENGTH_PENALTY = 0.7
NEG_INF_INIT = -1.0e8

N_CORES = 8
T_PER_CORE = T // N_CORES            # 4
ROWS = T_PER_CORE * B * K            # 640 rows per core
P = 128                              # partitions
CHUNK = 3142                         # 15 full chunks + tail of 3127
NCH = (V + CHUNK - 1) // CHUNK       # 16
NCAND = NCH * 8                      # 128 candidates per row (DVE path)

NTILES = ROWS // P                   # 5
POOL_TILES = 3                       # row-tiles handled by gpsimd topk
DVE_TILES = NTILES - POOL_TILES      # row-tiles handled by DVE max/max_index
DVE_ROWS = DVE_TILES * P             # 384
POOL_ROWS = POOL_TILES * P           # 256
NCALLS = POOL_ROWS // 8              # 32 topk calls (8 rows each)
VPAD = 50304                         # V padded to 16*3144 for topk layout
VSTRIPE = VPAD // 16                 # 3144
TOPK_K = 256
ESTRIDE = 32                         # one 128B granule per exp accumulator


def build_nc(dve_tiles=DVE_TILES, pool_tiles=POOL_TILES, v=V, chunk=CHUNK):
    nch = (v + chunk - 1) // chunk
    dve_rows = dve_tiles * P
    pool_rows = pool_tiles * P
    ncalls = pool_rows // 8
    nc = bacc.Bacc()
    x = nc.declare_dram_parameter(
        "x", [max(dve_rows, 1), v], mybir.dt.float32, isOutput=False)
    if pool_rows:
        xp = nc.declare_dram_parameter(
            "xp", [pool_rows, VPAD], mybir.dt.float32, isOutput=False)
    oidx = nc.declare_dram_parameter(
        "oidx", [max(dve_rows, 1), nch * 8], mybir.dt.uint32, isOutput=True)
    oes = nc.declare_dram_parameter(
        "oes", [max(dve_rows, 1), nch], mybir.dt.float32, isOutput=True)
    if pool_rows:
        otopk = nc.declare_dram_parameter(
            "otopk", [P, ncalls * 32], mybir.dt.uint32, isOutput=True)
        oesp = nc.declare_dram_parameter(
            "oesp", [P, ncalls], mybir.dt.float32, isOutput=True)

    roles = {}
    dma_exp = {}   # input dma -> its exp instruction
    slot_prev = {}  # input dma -> dma whose slot it reuses
    dve_hist, pool_hist = [], []
    with ExitStack() as octx:
        # gpsimd topk needs raw SBTensorHandles (not pool tiles)
        NRING = 4
        tk_in = [octx.enter_context(
            nc.sbuf_tensor(f"tkin{i}", [P, VSTRIPE], mybir.dt.float32))
            for i in range(NRING)] if pool_rows else []
        tk_out = octx.enter_context(nc.sbuf_tensor(
            "tkout", [P, max(ncalls, 1) * 32],
            mybir.dt.uint32)) if pool_rows else None

        with tile.TileContext(nc) as tc, ExitStack() as ctx:
            xpool = ctx.enter_context(tc.tile_pool(name="x", bufs=4))
            spool = ctx.enter_context(tc.tile_pool(name="scr", bufs=2))
            rpool = ctx.enter_context(tc.tile_pool(name="res", bufs=1))
            if pool_rows:
                nc.gpsimd.load_library(library_config.topk)

            # ---------------- DVE path ----------------
            vals8 = rpool.tile(
                [P, max(dve_tiles, 1) * nch * 8], mybir.dt.float32, tag="vals")
            idx8 = rpool.tile(
                [P, max(dve_tiles, 1) * nch * 8], mybir.dt.uint32, tag="idx")
            es = rpool.tile(
                [P, max(dve_tiles, 1) * nch * ESTRIDE], mybir.dt.float32,
                tag="es")
            esp = rpool.tile(
                [P, max(ncalls, 1) * ESTRIDE], mybir.dt.float32, tag="esp")
            # dense staging for the strided exp accumulators (a strided
            # 4B/128B DMA costs ~2.7us in descriptors; a DVE compact is free)
            esd = rpool.tile(
                [P, max(dve_tiles, 1) * nch + max(ncalls, 1)],
                mybir.dt.float32, tag="esd")

            def emit_dve(m, c):
                lo = c * chunk
                L = min(v, lo + chunk) - lo
                o8 = (m * nch + c) * 8
                oe = m * nch + c
                xt = xpool.tile([P, chunk], mybir.dt.float32, tag="xt")
                d = nc.sync.dma_start(
                    xt[:, :L], x[m * P:(m + 1) * P, lo:lo + L])
                mx = nc.vector.max(vals8[:, o8:o8 + 8], xt[:, :L])
                mi = nc.vector.max_index(
                    idx8[:, o8:o8 + 8], vals8[:, o8:o8 + 8], xt[:, :L])
                sc = spool.tile([P, 32], mybir.dt.float32, tag=f"sc_{m}_{c}")
                ex = nc.scalar.activation(
                    sc[:, 0:1].broadcast_to([P, L]), xt[:, :L],
                    mybir.ActivationFunctionType.Exp,
                    accum_out=es[:, oe * ESTRIDE:oe * ESTRIDE + 1])
                # chain only while the slot will be refilled (the chain is
                # what makes the refill DMA single-wait); tail chunks can
                # run exp in parallel with max/max_index
                if (m * nch + c) + 4 < dve_tiles * nch:
                    add_dep_helper(ex.ins, mi.ins, sync=True,
                                   reason="exp is last reader of chunk slot")
                roles[d.ins.name] = "dma_in"
                roles[mx.ins.name] = "max"
                roles[mi.ins.name] = "max_index"
                roles[ex.ins.name] = "exp"
                dma_exp[d.ins.name] = ex.ins.name
                dve_hist.append(d.ins.name)
                if len(dve_hist) > 4:
                    slot_prev[d.ins.name] = dve_hist[-5]
                if c == nch - 1:
                    b8 = m * nch * 8
                    o2 = nc.scalar.dma_start(
                        oidx[m * P:(m + 1) * P, :], idx8[:, b8:b8 + nch * 8])
                    be = m * nch * ESTRIDE
                    cp = nc.vector.tensor_copy(
                        esd[:, m * nch:(m + 1) * nch],
                        es[:, be:be + nch * ESTRIDE:ESTRIDE])
                    roles[o2.ins.name] = "dma_out_dve"
                    roles[cp.ins.name] = "escopy"
                    if m == dve_tiles - 1:
                        # single merged oes out for all dve tiles
                        dst = oes[:, :].rearrange("(m p) c -> p m c", p=P)
                        srcv = esd[:, :dve_tiles * nch].rearrange(
                            "p (m c) -> p m c", c=nch)
                        o3 = nc.scalar.dma_start(dst, srcv)
                        roles[o3.ins.name] = "dma_out_dve"

            def emit_pool(call):
                r0 = call * 8  # row offset within xp
                buf = tk_in[call % NRING]
                # one DMA: 8 padded rows -> [128, 3144], partition 16t+j
                # holds xp[r0+t, j*3144:(j+1)*3144]
                src = xp[r0:r0 + 8, :].rearrange("r (p c) -> (r p) c", p=16)
                d = nc.sync.dma_start(buf[:], src)
                tk = nc.gpsimd.topk(
                    tk_out[:, call * 32:(call + 1) * 32], buf[:],
                    tokens=8, vocab_size=VPAD, k=TOPK_K)
                sc = spool.tile([P, 32], mybir.dt.float32, tag=f"scp_{call}")
                ex = nc.scalar.activation(
                    sc[:, 0:1].broadcast_to([P, VSTRIPE]), buf[:],
                    mybir.ActivationFunctionType.Exp,
                    accum_out=esp[:, call * ESTRIDE:call * ESTRIDE + 1])
                if call + NRING < ncalls:
                    add_dep_helper(ex.ins, tk.ins, sync=True,
                                   reason="exp is last reader of topk slot")
                roles[d.ins.name] = "dma_in"
                roles[tk.ins.name] = "topk"
                roles[ex.ins.name] = "exp_pool"
                dma_exp[d.ins.name] = ex.ins.name
                pool_hist.append(d.ins.name)
                if len(pool_hist) > NRING:
                    slot_prev[d.ins.name] = pool_hist[-NRING - 1]
                half = ncalls // 2
                if call + 1 in (half, ncalls - 1, ncalls):
                    fl = {half: 0, ncalls - 1: half, ncalls: ncalls - 1}[call + 1]
                    o4 = nc.scalar.dma_start(
                        otopk[:, fl * 32:(call + 1) * 32],
                        tk_out[:, fl * 32:(call + 1) * 32])
                    roles[o4.ins.name] = "dma_out_pool"
                if call == ncalls - 1:
                    cp = nc.vector.tensor_copy(
                        esd[:, dve_tiles * nch:dve_tiles * nch + ncalls],
                        esp[:, :ncalls * ESTRIDE:ESTRIDE])
                    o5 = nc.scalar.dma_start(
                        oesp[:, :],
                        esd[:, dve_tiles * nch:dve_tiles * nch + ncalls])
                    roles[cp.ins.name] = "escopy"
                    roles[o5.ins.name] = "dma_out_dve"

            # interleave the two paths so DMA pressure and both compute
            # engines stay evenly fed (64 dve chunks : 48 pool calls = 4:3)
            dve_items = [(m, c) for m in range(dve_tiles) for c in range(nch)]
            pool_items = list(range(ncalls))
            di = pi = 0
            while di < len(dve_items) or pi < len(pool_items):
                for _ in range(4):
                    if di < len(dve_items):
                        emit_dve(*dve_items[di])
                        di += 1
                for _ in range(3):
                    if pi < len(pool_items):
                        emit_pool(pool_items[pi])
                        pi += 1

        # ---- single-wait legalization (see module docstring) ----
        _PREF = {"dma_in": ("Activation", "DMAHW", "DMASW", "DVE", "Pool"),
                 "max": ("DMAHW", "DMASW"),
                 "max_index": ("DVE",),
                 "topk": ("DMAHW", "DMASW"),
                 "exp": ("DVE",),
                 "exp_pool": ("Pool",),
                 "escopy": ("Activation",),
                 "dma_out_dve": ("DVE",),
                 "dma_out_pool": ("Pool",),
                 "dma_out_act": ("Activation",)}
        for blk in nc.m.functions[0].blocks:
            for inst in blk.instructions:
                si = inst.sync_info
                role = roles.get(inst.name)
                if si is None or role is None:
                    continue
                waits = list(si.on_wait)
                if len(waits) > 1:
                    for p in _PREF[role]:
                        kept = [w for w in waits if w.ant_name.startswith(p)]
                        if kept:
                            break
                    if len(kept) == 1:
                        inst.sync_info = mybir.SyncInfo(
                            on_wait=kept, on_update=list(si.on_update))

        # ---- queue-WAW coverage fixup ----
        # Tile assigns HWDGE queues in SCHEDULED order, so an input DMA's
        # same-queue predecessor is not statically known. Each input keeps
        # a single ACT wait; exp retirement at tick t implies every input
        # whose exp tick <= t has completed (exp is, or follows, the last
        # reader of its chunk). Raise each input's ACT wait to also cover
        # its queue predecessor's exp tick.
        act_tick = {}
        ticks = 0
        for blk in nc.m.functions[0].blocks:
            for inst in blk.instructions:
                if inst.engine == mybir.EngineType.Activation and \
                        inst.__class__.__name__ == "InstActivation":
                    ticks += 1
                    act_tick[inst.name] = ticks
        act_sem = None
        last_on_queue = {}
        for blk in nc.m.functions[0].blocks:
            for inst in blk.instructions:
                si = inst.sync_info
                if roles.get(inst.name) != "dma_in" or si is None:
                    continue
                q = si.on_update[0].ant_name
                prev = last_on_queue.get(q)
                last_on_queue[q] = inst.name
                aw = [w for w in si.on_wait
                      if w.ant_name.startswith("Activation")]
                if aw and act_sem is None:
                    act_sem = (aw[0].id, aw[0].ant_name)
                if prev is None:
                    continue
                need = act_tick[dma_exp[prev]]
                sp = slot_prev.get(inst.name)
                if sp is not None:
                    need = max(need, act_tick[dma_exp[sp]])
                have = aw[0].wait_value if aw else 0
                if need > have:
                    assert act_sem is not None
                    w = mybir.SyncWait(
                        sync_type="semaphore", id=act_sem[0],
                        ant_name=act_sem[1], wait_mode="sem-ge-imm",
                        wait_value=need, wait_reg=None)
                    inst.sync_info = mybir.SyncInfo(
                        on_wait=[w], on_update=list(si.on_update))
        nc.compile()
    return nc


_NC_CACHE = {}


def _get_nc():
    if "nc" not in _NC_CACHE:
        _NC_CACHE["nc"] = build_nc()
    return _NC_CACHE["nc"]


def _device_phase(logits):
    """Full logits [T,B,K,V] -> vals [T*B*K, NC] f32,
    gidx [T*B*K, NC] int64 (global vocab ids, NC=128, sorted desc),
    lse [T*B*K] f32."""
    nc = _get_nc()
    flat = logits.reshape(T * B * K, V)
    in_maps = []
    for c in range(N_CORES):
        block = flat[c * ROWS:(c + 1) * ROWS]
        xm = np.ascontiguousarray(block[:DVE_ROWS])
        xpm = np.full((POOL_ROWS, VPAD), -1.0e30, np.float32)
        xpm[:, :V] = block[DVE_ROWS:]
        in_maps.append({"x": xm, "xp": xpm})
    res = run_bass_kernel_spmd(nc, in_maps, list(range(N_CORES)))
    _NC_CACHE["last_results"] = res

    NC = NCAND
    nrows = T * B * K
    vals = np.empty((nrows, NC), np.float32)
    gidx = np.empty((nrows, NC), np.int64)
    esum = np.empty(nrows, np.float64)
    base = (np.arange(NCH, dtype=np.int64) * CHUNK)[None, :, None]
    for c in range(N_CORES):
        r = res.results[c]
        lo = c * ROWS
        # DVE rows: indices from device; values are exact gathers from
        # the host-resident logits (top-8 values are copies of inputs)
        lidx = r["oidx"].reshape(DVE_ROWS, NCH, 8).astype(np.int64) + base
        g = lidx.reshape(DVE_ROWS, NCH * 8)
        gidx[lo:lo + DVE_ROWS] = g
        blk = flat[lo:lo + DVE_ROWS]
        vals[lo:lo + DVE_ROWS] = np.take_along_axis(blk, g, axis=1)
        esum[lo:lo + DVE_ROWS] = r["oes"].astype(np.float64).sum(-1)
        # topk rows: otopk [128, ncalls*32]; token t of call -> partitions
        # 16t..16t+16, cols [call*32, call*32+16) vals / +16..32 idx,
        # globally ascending in row-major flatten order
        tk = r["otopk"].reshape(8, 16, NCALLS, 32).transpose(2, 0, 1, 3)
        # -> [call, tok, 16, 32]
        tvals = tk[:, :, :, :16].reshape(NCALLS, 8, 256).view(np.float32)
        tidx = tk[:, :, :, 16:].reshape(NCALLS, 8, 256).astype(np.int64)
        # top-NC, descending
        tvals = tvals[:, :, -NC:][:, :, ::-1].reshape(POOL_ROWS, NC)
        tidx = tidx[:, :, -NC:][:, :, ::-1].reshape(POOL_ROWS, NC)
        vals[lo + DVE_ROWS:lo + ROWS] = tvals
        gidx[lo + DVE_ROWS:lo + ROWS] = tidx
        ep = r["oesp"].reshape(8, 16, NCALLS).transpose(2, 0, 1)  # call,tok,16
        esum[lo + DVE_ROWS:lo + ROWS] = (
            ep.astype(np.float64).sum(-1).reshape(POOL_ROWS))
    lse = np.log(esum).astype(np.float32)
    return (vals.reshape(T, B, K, NC), gidx.reshape(T, B, K, NC),
            lse.reshape(T, B, K))


def _host_scan(vals, gidx, lse, eos_id):
    """Beam-search scan + backtrack, replicating the reference semantics."""
    flag = np.zeros((B, K), bool)
    score = np.full((B, K), NEG_INF_INIT, np.float32)
    score[:, 0] = 0.0
    now_length = np.zeros((B, K), np.float32)

    w_ids = np.empty((T, B, K), np.int64)
    regroups = np.empty((T, B, K), np.int64)
    eosmets = np.empty((T, B, K), bool)

    karange = np.arange(K)
    for t in range(T):
        valid = (~flag).astype(np.float32)                    # [B,K]
        w = vals[t] - lse[t][:, :, None]                      # [B,K,NC]
        cand = score[:, :, None] + w * valid[:, :, None]
        denom = ((now_length + valid + 1e-9) ** LENGTH_PENALTY
                 ).astype(np.float32)
        new_score = cand / denom[:, :, None]
        flat = karange[None, :, None] * V + gidx[t]           # [B,K,NC]

        # finished beams: only their v=0 candidate can be selected (the
        # min*10 penalty pushes v>=1 below every unpenalized candidate);
        # new_score = score/denom, cand = score there.
        ns = np.where(valid[:, :, None] > 0, new_score, -np.inf)
        cd = np.where(valid[:, :, None] > 0, cand, 0.0)
        synth_ns = np.where(valid > 0, -np.inf, score / denom)
        synth_cd = score
        synth_flat = np.broadcast_to(karange[None, :] * V, (B, K))

        ns_all = np.concatenate([ns.reshape(B, -1), synth_ns], axis=1)
        cd_all = np.concatenate([cd.reshape(B, -1), synth_cd], axis=1)
        fl_all = np.concatenate([flat.reshape(B, -1), synth_flat], axis=1)

        sel = np.empty((B, K), np.int64)
        for b in range(B):
            order = np.lexsort((fl_all[b], -ns_all[b]))  # value desc, idx asc
            sel[b] = order[:K]
        score = np.take_along_axis(cd_all, sel, axis=1).astype(np.float32)
        flat_sel = np.take_along_axis(fl_all, sel, axis=1)

        regroup = flat_sel // V
        w_id = flat_sel % V
        flag_g = np.take_along_axis(flag, regroup, axis=1)
        now_length = np.take_along_axis(now_length, regroup, axis=1) + valid
        eosmets[t] = flag_g
        flag = flag_g | ((w_id == eos_id) & (valid > 0.0))
        w_ids[t] = w_id
        regroups[t] = regroup

    # backtrack (reverse gather chain)
    now_index = np.tile(np.arange(K)[None, :], (B, 1))
    wo = np.empty((T, B, K), np.int64)
    em = np.empty((T, B, K), bool)
    for t in range(T - 1, -1, -1):
        wo[t] = np.take_along_axis(w_ids[t], now_index, axis=1)
        em[t] = np.take_along_axis(eosmets[t], now_index, axis=1)
        now_index = np.take_along_axis(regroups[t], now_index, axis=1)
    back_eosmet = 1 - em.astype(np.int32)
    wo = (wo * back_eosmet).astype(np.int32)
    length = back_eosmet.sum(axis=0).astype(np.int32)
    return wo, length, score


def kernel(logits, eos_id):
    logits = np.ascontiguousarray(np.asarray(logits, dtype=np.float32))
    assert logits.shape == (T, B, K, V), logits.shape
    eos = int(np.asarray(eos_id))
    vals, gidx, lse = _device_phase(logits)
    return _host_scan(vals, gidx, lse, eos)
